# revision 4
# baseline (speedup 1.0000x reference)
"""Trainium2 Bass kernel for nn_Attention_38182259261827.

Multi-head attention (B=4, C=512, L=2048, H=8, D=64) with pointwise-conv
QKV / output projections on 8 NeuronCores (SPMD, no collectives).

Sharding: core c handles batch b=c//2, head-group g=c%2 (4 heads each).
Inputs are sharded, pre-cast to bf16, AND pre-packed host-side into
[128, chunks, free] layouts so each input lands in 2 large DMAs (one per
HWDGE queue) instead of 15 small ones; each core computes its partial
output-projection sum over its 4 heads; the two bf16 partials per batch
are summed host-side in f32 (plus bias).

Per-core pipeline (matmuls bf16, f32 PSUM accumulation):
  1. 10 PE warmup matmuls un-throttle the HAM clock gate while the
     packed inputs land (~7us); projection chains then run at full
     clock.
  2. qk projection into per-(m,lt) [128,512] tiles so the first strip's
     S^T only waits on the k/q blocks it actually reads; prologue chain
     copies ride the still-idle ACT, steady-state filler copies ride
     DVE (kept shallow so PSUM st slots release fast).
  3. v^T projection: va[l, 4, 65] = per-head [v^T | ones]; the ones
     column (strided memset) makes PV emit softmax row-sums for free.
  4. Per (i-tile 512, head) strip: S^T[j, i] = k^T q on PE (K=64);
     exp(S*scale) on ACT over j-chunk groups [2,3,3,3,3,2]; PV with va
     stationary accumulates O^T[65, i] in PSUM (row 64 = denominators).
     S^T/exp leads PV by two groups; leftover projection chains feed in
     as PE fillers. Normalize: DVE reciprocal + stride-0 DMA partition
     broadcast + DVE multiply into O^T sbuf.
  5. Output projections in [128,2,512] PSUM pair-tiles: the it=0..2
     chains are emitted inside the LAST strip's iterations (their norms
     are long done) so they fill the PE idle of the exp/PV tail; only
     it=3 trails the final norm. Whole-tile copies alternate DVE/ACT,
     bf16 output DMAs alternate both HWDGE queues.

PSUM: st 2x3 banks + o 2x1 = 8; qk/v/drain chains reuse the st tag.

Hard-won cautions: (1) SBUF allocation order is load-bearing - layout
shifts cost ~20% on ALL engines (bank conflicts); (2) PSUM st/o
rotation readers must clear promptly - pace DVE work so copies/norm ops
never queue deep (a stalled norm stalls the o rotation and re-throttles
the PE clock); (3) HAM down-clocks after >3.4us PE idle - warmup must
bridge until the first projection chains and the drain MM stream must
stay dense.
"""

import sys

if "/opt/trn_rl_repo" not in sys.path:
    sys.path.insert(0, "/opt/trn_rl_repo")

import numpy as np
import ml_dtypes

import concourse.bass as bass
import concourse.mybir as mybir
from concourse import bacc
from concourse.tile import TileContext
from concourse.bass_utils import run_bass_kernel_spmd

F32 = mybir.dt.float32
BF16 = mybir.dt.bfloat16
EXP = mybir.ActivationFunctionType.Exp

B, C, L = 4, 512, 2048
HEADS, D = 8, 64
HL = HEADS // 2          # 4 local heads per core
SCALE = D ** -0.5        # 0.125
N_CORES = 8
NKC = C // 128           # 4 contraction chunks
NLT = L // 512           # 4 l/i tiles of 512
NLC = L // 128           # 16 l/j chunks of 128
J_GROUPS = [[0, 1], [2, 3, 4], [5, 6, 7], [8, 9, 10], [11, 12, 13], [14, 15]]
N_WARMUP = 10            # PE pre-warm matmuls (HAM un-throttle)

_NC_CACHE = None


def _build_nc():
    nc = bacc.Bacc(
        "TRN2",
        target_bir_lowering=False,
        debug=False,
        enable_asserts=False,
        num_devices=N_CORES,
    )
    # Packed DRAM layouts: [128, kc, free] with kc = C//128 row-chunks.
    x_d = nc.dram_tensor("x", [128, NKC, L], BF16, kind="ExternalInput")
    wqk_d = nc.dram_tensor("wqk", [128, NKC, 512], BF16, kind="ExternalInput")
    wv_d = nc.dram_tensor("wv", [128, NKC, 256], BF16, kind="ExternalInput")
    wo_d = nc.dram_tensor("wout", [128, 2, C], BF16, kind="ExternalInput")
    out_d = nc.dram_tensor("out", [C, L], BF16, kind="ExternalOutput")

    with TileContext(nc) as tc:
        with (
            tc.tile_pool(name="sb", bufs=1) as SB,
            tc.tile_pool(name="ps", space="PSUM", bufs=1) as PS,
        ):
            warm = SB.tile([128, 512], BF16, tag="warm", bufs=1, name="warm")
            nc.vector.memset(warm, 0.0)
            for w in range(N_WARMUP):
                wp = PS.tile([128, 512], F32, tag="o", bufs=2, name=f"wp{w}")
                nc.tensor.matmul(wp, warm[:, 0:128], warm, start=True, stop=True)

            wqk_all = SB.tile([128, NKC, 512], BF16, tag="wqk", bufs=1, name="wqk")
            wv_all = SB.tile([128, NKC, 256], BF16, tag="wv", bufs=1, name="wv")
            wo_all = SB.tile([128, 2, 512], BF16, tag="wo", bufs=1, name="wo")
            x_all = SB.tile([128, NKC, L], BF16, tag="x", bufs=1, name="x")
            # 2 large DMAs per input, split across both HWDGE queues;
            # weights that gate the first chains lead.
            nc.sync.dma_start(out=wqk_all[:, 0:2, :], in_=wqk_d[:, 0:2, :])
            nc.scalar.dma_start(out=wqk_all[:, 2:4, :], in_=wqk_d[:, 2:4, :])
            nc.sync.dma_start(out=x_all[:, 0:2, :], in_=x_d[:, 0:2, :])
            nc.scalar.dma_start(out=x_all[:, 2:4, :], in_=x_d[:, 2:4, :])
            nc.sync.dma_start(out=wv_all[:, 0:2, :], in_=wv_d[:, 0:2, :])
            nc.scalar.dma_start(out=wv_all[:, 2:4, :], in_=wv_d[:, 2:4, :])
            nc.sync.dma_start(out=wo_all[:, 0:1, :], in_=wo_d[:, 0:1, :])
            nc.scalar.dma_start(out=wo_all[:, 1:2, :], in_=wo_d[:, 1:2, :])

            # qk projection output: per-(m, lt) [128, 512] tiles.
            # m=0,1 -> q of head pairs (0,1),(2,3); m=2,3 -> k likewise.
            qk_bf = [
                [
                    SB.tile([128, 512], BF16, tag=f"qk{m}_{lt}", bufs=1,
                            name=f"qk{m}_{lt}")
                    for lt in range(NLT)
                ]
                for m in range(4)
            ]

            def qk_chain(m, lt, on_act=False):
                pp = PS.tile([128, 512], F32, tag="st", bufs=2, name=f"ppqk{m}_{lt}")
                for kc in range(NKC):
                    nc.tensor.matmul(
                        pp,
                        wqk_all[:, kc, m * 128:(m + 1) * 128],
                        x_all[:, kc, lt * 512:(lt + 1) * 512],
                        start=(kc == 0),
                        stop=(kc == NKC - 1),
                    )
                if on_act:
                    nc.scalar.copy(qk_bf[m][lt], pp)
                else:
                    nc.vector.tensor_copy(qk_bf[m][lt], pp)

            va = SB.tile([128, 4 * NLC, 65], BF16, tag="va", bufs=1, name="va")
            nc.vector.memset(va[:, :, 64:65], 1.0)

            def v_chain(lc):
                vp = PS.tile([128, 256], F32, tag="st", bufs=2, name=f"vp{lc}")
                for kc in range(NKC):
                    nc.tensor.matmul(
                        vp,
                        x_all[:, kc, lc * 128:(lc + 1) * 128],
                        wv_all[:, kc, :],
                        start=(kc == 0),
                        stop=(kc == NKC - 1),
                    )
                nc.vector.tensor_copy(
                    va[:, lc * 4:(lc + 1) * 4, 0:64],
                    vp.rearrange("p (h d) -> p h d", h=4),
                )

            # Prologue: exactly what strip (0,0)/(0,1) need first. Copies on
            # the still-idle ACT so DVE stays clear for v casts.
            qk_chain(2, 0, on_act=True)
            qk_chain(0, 0, on_act=True)
            v_chain(0)
            v_chain(1)
            qk_chain(2, 1, on_act=True)
            v_chain(2)
            v_chain(3)
            qk_chain(2, 2, on_act=True)
            v_chain(4)
            v_chain(5)
            qk_chain(2, 3, on_act=True)

            # per-i-tile OT tiles: OT deps are tracked whole-tile, so a
            # single [128, L] tile makes every drain out-chain wait the
            # LAST normalize; per-it tiles scope each chain to its own
            # i-tile's norms. Same bytes/alloc position, no SBUF shift.
            OT_bf = [
                [
                    SB.tile([128, 512], BF16, tag=f"otb{i}_{it}", bufs=1,
                            name=f"otb{i}_{it}")
                    for it in range(NLT)
                ]
                for i in range(2)
            ]

            strips = [(it, h) for it in range(NLT) for h in range(HL)]
            o_tiles = {}

            def emit_st(it, h, grp, gi0):
                qp = 64 * (h % 2)
                q_ap = qk_bf[h // 2][it][qp:qp + 64, :]
                st = PS.tile(
                    [128, 3, 512], F32, tag="st", bufs=2, name=f"st{it}_{h}_{gi0}"
                )
                for gi, jc in enumerate(grp):
                    k_ap = qk_bf[2 + h // 2][jc // 4][qp:qp + 64,
                                                      (jc % 4) * 128:(jc % 4 + 1) * 128]
                    nc.tensor.matmul(st[:, gi, :], k_ap, q_ap, start=True, stop=True)
                g = len(grp)
                ex = SB.tile(
                    [128, 3, 512], BF16, tag="exp", bufs=4, name=f"ex{it}_{h}_{gi0}"
                )
                nc.scalar.activation(
                    ex[:, 0:g, :], st[:, 0:g, :], EXP, scale=float(SCALE)
                )
                return ex

            def emit_pv(it, h, grp, ex):
                o_ps = o_tiles[(it, h)]
                for gi, jc in enumerate(grp):
                    nc.tensor.matmul(
                        o_ps,
                        va[:, jc * 4 + h, :],
                        ex[:, gi, :],
                        start=(jc == 0),
                        stop=(jc == NLC - 1),
                    )

            def emit_norm(it, h):
                o_ps = o_tiles.pop((it, h))
                rsum = SB.tile([1, 512], F32, tag="rsum", bufs=2, name=f"rsum{it}_{h}")
                nc.vector.tensor_copy(rsum, o_ps[64:65, :])
                rs = SB.tile([1, 512], F32, tag="rs", bufs=2, name=f"rs{it}_{h}")
                nc.vector.reciprocal_approx_fast(rs, rsum)
                rb = SB.tile([64, 512], F32, tag="rb", bufs=2, name=f"rb{it}_{h}")
                rs_rep = bass.AP(
                    tensor=rs.tensor,
                    offset=rs.offset,
                    ap=[[1, 1], [0, 64], [1, 512]],
                )
                nc.sync.dma_start(out=rb, in_=rs_rep)
                cp = 64 * (h % 2)
                nc.vector.tensor_mul(
                    OT_bf[h // 2][it][cp:cp + 64, :],
                    o_ps[0:64, :],
                    rb,
                )

            ysb_k = [0]

            def out_pair(it, fp):
                yp = PS.tile([128, 2, 512], F32, tag="st", bufs=2,
                             name=f"yp{it}_{fp}")
                for sub in range(2):
                    fm = 2 * fp + sub
                    for kc in range(2):
                        nc.tensor.matmul(
                            yp[:, sub, :],
                            wo_all[:, kc, fm * 128:(fm + 1) * 128],
                            OT_bf[kc][it],
                            start=(kc == 0),
                            stop=(kc == 1),
                        )
                ysb = SB.tile([128, 2, 512], BF16, tag="ysb", bufs=4,
                              name=f"ysb{it}_{fp}")
                if ysb_k[0] % 2 == 0:
                    nc.vector.tensor_copy(ysb, yp)
                else:
                    nc.scalar.copy(ysb, yp)
                ysb_k[0] += 1
                for sub in range(2):
                    fm = 2 * fp + sub
                    eng = nc.sync if fm % 2 == 0 else nc.scalar
                    eng.dma_start(
                        out=out_d[fm * 128:(fm + 1) * 128,
                                  it * 512:(it + 1) * 512],
                        in_=ysb[:, sub, :],
                    )

            flat = [
                (it, h, grp, gi0)
                for (it, h) in strips
                for gi0, grp in enumerate(J_GROUPS)
            ]
            filler_q = [
                ("v", 6), ("v", 7), ("v", 8), ("v", 9), ("v", 10), ("v", 11),
                ("v", 12), ("v", 13), ("v", 14), ("v", 15),
                (3, 0), (3, 1), (3, 2), (3, 3), (1, 0),
                (0, 1), (1, 1), (0, 2), (1, 2), (0, 3), (1, 3),
            ]
            from collections import deque
            exq = deque()

            def lead_alloc(idx):
                it, h, grp, gi0 = flat[idx]
                if gi0 == 0:
                    o_tiles[(it, h)] = PS.tile(
                        [65, 512], F32, tag="o", bufs=2, name=f"o{it}_{h}"
                    )
                exq.append((it, h, grp, emit_st(it, h, grp, gi0)))

            lead_alloc(0)
            lead_alloc(1)
            n_flat = len(flat)
            for fi in range(n_flat):
                if fi + 2 < n_flat:
                    lead_alloc(fi + 2)
                n_pop = 2 if fi < 8 else 1
                for _ in range(n_pop):
                    if filler_q:
                        kind, a = filler_q.pop(0)
                        if kind == "v":
                            v_chain(a)
                        else:
                            qk_chain(kind, a)
                pit, ph, pgrp, pex = exq.popleft()
                emit_pv(pit, ph, pgrp, pex)
                if pgrp is J_GROUPS[-1]:
                    emit_norm(pit, ph)
                # it=0..2 output chains only depend on long-finished norms:
                # emit them through the LAST strip's iterations to fill the
                # PE idle of the exp/PV tail.
                if fi >= n_flat - 6:
                    k = fi - (n_flat - 6)
                    out_pair(k // 2, k % 2)
            out_pair(3, 0)
            out_pair(3, 1)
    nc.compile()
    return nc


def _shard_inputs(x, w_qkv, w_out):
    bf = ml_dtypes.bfloat16

    def pack(t, nch):
        # [nch*128, F] -> [128, nch, F]
        f = t.shape[1]
        return np.ascontiguousarray(
            t.reshape(nch, 128, f).transpose(1, 0, 2).astype(bf)
        )

    in_maps = []
    for c in range(N_CORES):
        b, g = c // 2, c % 2
        cols = slice(g * 256, (g + 1) * 256)
        wqk_c = np.concatenate(
            [w_qkv[:, 0:512][:, cols], w_qkv[:, 512:1024][:, cols]], axis=1
        )
        wv_c = w_qkv[:, 1024:1536][:, cols]
        wo_c = w_out[g * 256:(g + 1) * 256, :]
        in_maps.append(
            {
                "x": pack(x[b], NKC),
                "wqk": pack(wqk_c, NKC),
                "wv": pack(wv_c, NKC),
                "wout": pack(wo_c, 2),
            }
        )
    return in_maps


def _run(x, w_qkv, w_out, b_out, trace=False, tmpdir=None):
    global _NC_CACHE
    if _NC_CACHE is None:
        _NC_CACHE = _build_nc()
    nc = _NC_CACHE
    in_maps = _shard_inputs(
        np.asarray(x, np.float32),
        np.asarray(w_qkv, np.float32),
        np.asarray(w_out, np.float32),
    )
    res = run_bass_kernel_spmd(
        nc, in_maps, core_ids=list(range(N_CORES)), trace=trace, tmpdir=tmpdir
    )
    b_out = np.asarray(b_out, np.float32)
    y = np.empty((B, C, L), np.float32)
    for b in range(B):
        y[b] = (
            res.results[2 * b]["out"].astype(np.float32)
            + res.results[2 * b + 1]["out"].astype(np.float32)
            + b_out[:, None]
        )
    return y, res


def kernel(x, w_qkv, w_out, b_out):
    y, _ = _run(x, w_qkv, w_out, b_out, trace=False)
    return y


# revision 5
# speedup vs baseline: 1.0197x; 1.0197x over previous
"""Trainium2 Bass kernel for nn_Attention_38182259261827.

Multi-head attention (B=4, C=512, L=2048, H=8, D=64) with pointwise-conv
QKV / output projections on 8 NeuronCores (SPMD, no collectives).

Sharding: core c handles batch b=c//2, head-group g=c%2 (4 heads each).
Inputs are sharded, pre-cast to bf16, AND pre-packed host-side into
[128, chunks, free] layouts so each input lands in few large DMAs split
across both HWDGE queues; each core computes its partial output-
projection sum over its 4 heads; the two bf16 partials per batch are
summed host-side in f32 (plus bias).

Per-core pipeline (matmuls bf16, f32 PSUM accumulation):
  1. Ramp: wqk lands first (~3.5us), x chunks follow (x0/x1 ~9us,
     x2/x3 ~14us). The first two projection chains are SPLIT around the
     x2/x3 wait: their kc=0/1 matmuls run as soon as x0/x1 land, six
     warmup matmuls bridge the gap to x2/x3 (no >3.4us PE idle, so the
     HAM clock gate stays open), and the kc=2/3 halves + copies complete
     ~14.5us -> first exp ~16us.
  2. qk projection into per-(m,lt) [128,512] tiles so the first strip's
     S^T only waits on the k/q blocks it actually reads. qk-proj copies
     ride ACT (in-order behind exps, so PSUM st slots release promptly -
     putting them on DVE queues them behind norm ops and stalls the o
     rotation into a HAM re-throttle spiral). v-proj copies ride DVE.
  3. v^T projection: va[l, 4, 65] = per-head [v^T | ones]; the ones
     column (strided memset) makes PV emit softmax row-sums for free.
  4. Per (i-tile 512, head) strip: S^T[j, i] = k^T q on PE (K=64);
     exp(S*scale) on ACT over j-chunk groups [2,3,3,3,3,2]; PV with va
     stationary accumulates O^T[65, i] in PSUM (row 64 = denominators).
     S^T/exp leads PV by two groups; leftover projection chains feed in
     as PE fillers. Normalize: DVE reciprocal + stride-0 DMA partition
     broadcast + DVE multiply into O^T sbuf.
  5. Output projections in [128,2,512] PSUM pair-tiles: the it=0..2
     chains are emitted inside the LAST strip's iterations (their norms
     are long done) so they fill the PE idle of the exp/PV tail, with
     copies on DVE (ACT is still the exp pacer there); only it=3 trails
     the final norm, with copies on the by-then-idle ACT. bf16 output
     DMAs alternate both HWDGE queues.

PSUM: st 2x3 banks + o 2x1 = 8; qk/v/drain chains reuse the st tag.

Hard-won cautions: (1) SBUF allocation order is load-bearing - layout
shifts cost ~20% on ALL engines (bank conflicts); (2) PSUM st/o
rotation readers must clear promptly - a stalled DVE queue stalls the o
rotation and re-throttles the PE clock; (3) HAM down-clocks after
>3.4us PE idle - bridge every DMA wait with real or warmup matmuls and
keep the drain MM stream dense.
"""

import sys

if "/opt/trn_rl_repo" not in sys.path:
    sys.path.insert(0, "/opt/trn_rl_repo")

import numpy as np
import ml_dtypes

import concourse.bass as bass
import concourse.mybir as mybir
from concourse import bacc
from concourse.tile import TileContext
from concourse.bass_utils import run_bass_kernel_spmd

F32 = mybir.dt.float32
BF16 = mybir.dt.bfloat16
EXP = mybir.ActivationFunctionType.Exp

B, C, L = 4, 512, 2048
HEADS, D = 8, 64
HL = HEADS // 2          # 4 local heads per core
SCALE = D ** -0.5        # 0.125
N_CORES = 8
NKC = C // 128           # 4 contraction chunks
NLT = L // 512           # 4 l/i tiles of 512
NLC = L // 128           # 16 l/j chunks of 128
J_GROUPS = [[0, 1], [2, 3, 4], [5, 6, 7], [8, 9, 10], [11, 12, 13], [14, 15]]

_NC_CACHE = None


def _build_nc():
    nc = bacc.Bacc(
        "TRN2",
        target_bir_lowering=False,
        debug=False,
        enable_asserts=False,
        num_devices=N_CORES,
    )
    # Packed DRAM layouts: [128, kc, free] with kc = C//128 row-chunks.
    x_d = nc.dram_tensor("x", [128, NKC, L], BF16, kind="ExternalInput")
    wqk_d = nc.dram_tensor("wqk", [128, NKC, 512], BF16, kind="ExternalInput")
    wv_d = nc.dram_tensor("wv", [128, NKC, 256], BF16, kind="ExternalInput")
    wo_d = nc.dram_tensor("wout", [128, 2, C], BF16, kind="ExternalInput")
    out_d = nc.dram_tensor("out", [C, L], BF16, kind="ExternalOutput")

    with TileContext(nc) as tc:
        with (
            tc.tile_pool(name="sb", bufs=1) as SB,
            tc.tile_pool(name="ps", space="PSUM", bufs=1) as PS,
        ):
            warm = SB.tile([128, 512], BF16, tag="warm", bufs=1, name="warm")
            nc.vector.memset(warm, 0.0)
            n_wp = [0]

            def warmup(n):
                for _ in range(n):
                    wp = PS.tile([128, 512], F32, tag="o", bufs=2,
                                 name=f"wp{n_wp[0]}")
                    n_wp[0] += 1
                    nc.tensor.matmul(wp, warm[:, 0:128], warm, start=True,
                                     stop=True)

            warmup(2)

            wqk_all = SB.tile([128, NKC, 512], BF16, tag="wqk", bufs=1, name="wqk")
            wv_all = SB.tile([128, NKC, 256], BF16, tag="wv", bufs=1, name="wv")
            wo_all = SB.tile([128, 2, 512], BF16, tag="wo", bufs=1, name="wo")
            x_bf = [
                SB.tile([128, L], BF16, tag=f"x{i}", bufs=1, name=f"x{i}")
                for i in range(NKC)
            ]
            # wqk first (gates the split prologue chains), then x chunks,
            # then wv/wo - all split across both HWDGE queues.
            nc.sync.dma_start(out=wqk_all[:, 0:2, :], in_=wqk_d[:, 0:2, :])
            nc.scalar.dma_start(out=wqk_all[:, 2:4, :], in_=wqk_d[:, 2:4, :])
            nc.sync.dma_start(out=x_bf[0], in_=x_d[:, 0, :])
            nc.scalar.dma_start(out=x_bf[1], in_=x_d[:, 1, :])
            nc.sync.dma_start(out=x_bf[2], in_=x_d[:, 2, :])
            nc.scalar.dma_start(out=x_bf[3], in_=x_d[:, 3, :])
            nc.sync.dma_start(out=wv_all[:, 0:2, :], in_=wv_d[:, 0:2, :])
            nc.scalar.dma_start(out=wv_all[:, 2:4, :], in_=wv_d[:, 2:4, :])
            nc.sync.dma_start(out=wo_all[:, 0:1, :], in_=wo_d[:, 0:1, :])
            nc.scalar.dma_start(out=wo_all[:, 1:2, :], in_=wo_d[:, 1:2, :])

            # qk projection output: per-(m, lt) [128, 512] tiles.
            # m=0,1 -> q of head pairs (0,1),(2,3); m=2,3 -> k likewise.
            qk_bf = [
                [
                    SB.tile([128, 512], BF16, tag=f"qk{m}_{lt}", bufs=1,
                            name=f"qk{m}_{lt}")
                    for lt in range(NLT)
                ]
                for m in range(4)
            ]

            def qk_mms(pp, m, lt, kcs, start, stop):
                for kc in kcs:
                    nc.tensor.matmul(
                        pp,
                        wqk_all[:, kc, m * 128:(m + 1) * 128],
                        x_bf[kc][:, lt * 512:(lt + 1) * 512],
                        start=(start and kc == kcs[0]),
                        stop=(stop and kc == kcs[-1]),
                    )

            def qk_chain(m, lt):
                pp = PS.tile([128, 512], F32, tag="st", bufs=2, name=f"ppqk{m}_{lt}")
                qk_mms(pp, m, lt, [0, 1, 2, 3], True, True)
                nc.scalar.copy(qk_bf[m][lt], pp)

            # Split prologue pair: kc0/1 halves run when x0/x1 land (~9us),
            # warmup bridges to x2/x3 (~14us), then kc2/3 + copies.
            pp_a = PS.tile([128, 512], F32, tag="st", bufs=2, name="pp_a")
            pp_b = PS.tile([128, 512], F32, tag="st", bufs=2, name="pp_b")
            qk_mms(pp_a, 2, 0, [0, 1], True, False)
            qk_mms(pp_b, 0, 0, [0, 1], True, False)
            warmup(6)
            qk_mms(pp_a, 2, 0, [2, 3], False, True)
            nc.scalar.copy(qk_bf[2][0], pp_a)
            qk_mms(pp_b, 0, 0, [2, 3], False, True)
            nc.scalar.copy(qk_bf[0][0], pp_b)

            va = SB.tile([128, 4 * NLC, 65], BF16, tag="va", bufs=1, name="va")
            nc.vector.memset(va[:, :, 64:65], 1.0)

            def v_chain(lc):
                vp = PS.tile([128, 256], F32, tag="st", bufs=2, name=f"vp{lc}")
                for kc in range(NKC):
                    nc.tensor.matmul(
                        vp,
                        x_bf[kc][:, lc * 128:(lc + 1) * 128],
                        wv_all[:, kc, :],
                        start=(kc == 0),
                        stop=(kc == NKC - 1),
                    )
                nc.vector.tensor_copy(
                    va[:, lc * 4:(lc + 1) * 4, 0:64],
                    vp.rearrange("p (h d) -> p h d", h=4),
                )

            qk_chain(2, 1)
            v_chain(0)
            v_chain(1)
            qk_chain(2, 2)
            qk_chain(2, 3)

            # per-i-tile OT tiles: OT deps are tracked whole-tile, so a
            # single [128, L] tile makes every drain out-chain wait the
            # LAST normalize; per-it tiles scope each chain to its own
            # i-tile's norms. Same bytes/alloc position, no SBUF shift.
            OT_bf = [
                [
                    SB.tile([128, 512], BF16, tag=f"otb{i}_{it}", bufs=1,
                            name=f"otb{i}_{it}")
                    for it in range(NLT)
                ]
                for i in range(2)
            ]

            strips = [(it, h) for it in range(NLT) for h in range(HL)]
            o_tiles = {}

            def emit_st(it, h, grp, gi0):
                qp = 64 * (h % 2)
                q_ap = qk_bf[h // 2][it][qp:qp + 64, :]
                st = PS.tile(
                    [128, 3, 512], F32, tag="st", bufs=2, name=f"st{it}_{h}_{gi0}"
                )
                for gi, jc in enumerate(grp):
                    k_ap = qk_bf[2 + h // 2][jc // 4][qp:qp + 64,
                                                      (jc % 4) * 128:(jc % 4 + 1) * 128]
                    nc.tensor.matmul(st[:, gi, :], k_ap, q_ap, start=True, stop=True)
                g = len(grp)
                ex = SB.tile(
                    [128, 3, 512], BF16, tag="exp", bufs=4, name=f"ex{it}_{h}_{gi0}"
                )
                nc.scalar.activation(
                    ex[:, 0:g, :], st[:, 0:g, :], EXP, scale=float(SCALE)
                )
                return ex

            def emit_pv(it, h, grp, ex):
                o_ps = o_tiles[(it, h)]
                for gi, jc in enumerate(grp):
                    nc.tensor.matmul(
                        o_ps,
                        va[:, jc * 4 + h, :],
                        ex[:, gi, :],
                        start=(jc == 0),
                        stop=(jc == NLC - 1),
                    )

            def emit_norm(it, h):
                o_ps = o_tiles.pop((it, h))
                rsum = SB.tile([1, 512], F32, tag="rsum", bufs=2, name=f"rsum{it}_{h}")
                nc.vector.tensor_copy(rsum, o_ps[64:65, :])
                rs = SB.tile([1, 512], F32, tag="rs", bufs=2, name=f"rs{it}_{h}")
                nc.vector.reciprocal_approx_fast(rs, rsum)
                rb = SB.tile([64, 512], F32, tag="rb", bufs=2, name=f"rb{it}_{h}")
                rs_rep = bass.AP(
                    tensor=rs.tensor,
                    offset=rs.offset,
                    ap=[[1, 1], [0, 64], [1, 512]],
                )
                nc.sync.dma_start(out=rb, in_=rs_rep)
                cp = 64 * (h % 2)
                nc.vector.tensor_mul(
                    OT_bf[h // 2][it][cp:cp + 64, :],
                    o_ps[0:64, :],
                    rb,
                )

            def out_pair(it, fp, on_act):
                yp = PS.tile([128, 2, 512], F32, tag="st", bufs=2,
                             name=f"yp{it}_{fp}")
                for sub in range(2):
                    fm = 2 * fp + sub
                    for kc in range(2):
                        nc.tensor.matmul(
                            yp[:, sub, :],
                            wo_all[:, kc, fm * 128:(fm + 1) * 128],
                            OT_bf[kc][it],
                            start=(kc == 0),
                            stop=(kc == 1),
                        )
                ysb = SB.tile([128, 2, 512], BF16, tag="ysb", bufs=4,
                              name=f"ysb{it}_{fp}")
                if on_act:
                    nc.scalar.copy(ysb, yp)
                else:
                    nc.vector.tensor_copy(ysb, yp)
                for sub in range(2):
                    fm = 2 * fp + sub
                    eng = nc.sync if fm % 2 == 0 else nc.scalar
                    eng.dma_start(
                        out=out_d[fm * 128:(fm + 1) * 128,
                                  it * 512:(it + 1) * 512],
                        in_=ysb[:, sub, :],
                    )

            flat = [
                (it, h, grp, gi0)
                for (it, h) in strips
                for gi0, grp in enumerate(J_GROUPS)
            ]
            filler_q = [
                ("v", 2), ("v", 3), ("v", 4), ("v", 5), ("v", 6), ("v", 7),
                ("v", 8), ("v", 9), ("v", 10), ("v", 11), ("v", 12), ("v", 13),
                ("v", 14), ("v", 15),
                (3, 0), (3, 1), (3, 2), (3, 3), (1, 0),
                (0, 1), (1, 1), (0, 2), (1, 2), (0, 3), (1, 3),
            ]
            from collections import deque
            exq = deque()

            def lead_alloc(idx):
                it, h, grp, gi0 = flat[idx]
                if gi0 == 0:
                    o_tiles[(it, h)] = PS.tile(
                        [65, 512], F32, tag="o", bufs=2, name=f"o{it}_{h}"
                    )
                exq.append((it, h, grp, emit_st(it, h, grp, gi0)))

            lead_alloc(0)
            lead_alloc(1)
            n_flat = len(flat)
            for fi in range(n_flat):
                if fi + 2 < n_flat:
                    lead_alloc(fi + 2)
                n_pop = 3 if fi < 4 else (2 if fi < 10 else 1)
                for _ in range(n_pop):
                    if filler_q:
                        kind, a = filler_q.pop(0)
                        if kind == "v":
                            v_chain(a)
                        else:
                            qk_chain(kind, a)
                pit, ph, pgrp, pex = exq.popleft()
                emit_pv(pit, ph, pgrp, pex)
                if pgrp is J_GROUPS[-1]:
                    emit_norm(pit, ph)
                # it=0..2 output chains only depend on long-finished norms:
                # emit them through the LAST strip's iterations to fill the
                # PE idle of the exp/PV tail (copies on DVE - ACT is still
                # the exp pacer there).
                if fi >= n_flat - 6:
                    k = fi - (n_flat - 6)
                    out_pair(k // 2, k % 2, on_act=False)
            out_pair(3, 0, on_act=True)
            out_pair(3, 1, on_act=True)
    nc.compile()
    return nc


def _shard_inputs(x, w_qkv, w_out):
    bf = ml_dtypes.bfloat16

    def pack(t, nch):
        # [nch*128, F] -> [128, nch, F]
        f = t.shape[1]
        return np.ascontiguousarray(
            t.reshape(nch, 128, f).transpose(1, 0, 2).astype(bf)
        )

    in_maps = []
    for c in range(N_CORES):
        b, g = c // 2, c % 2
        cols = slice(g * 256, (g + 1) * 256)
        wqk_c = np.concatenate(
            [w_qkv[:, 0:512][:, cols], w_qkv[:, 512:1024][:, cols]], axis=1
        )
        wv_c = w_qkv[:, 1024:1536][:, cols]
        wo_c = w_out[g * 256:(g + 1) * 256, :]
        in_maps.append(
            {
                "x": pack(x[b], NKC),
                "wqk": pack(wqk_c, NKC),
                "wv": pack(wv_c, NKC),
                "wout": pack(wo_c, 2),
            }
        )
    return in_maps


def _run(x, w_qkv, w_out, b_out, trace=False, tmpdir=None):
    global _NC_CACHE
    if _NC_CACHE is None:
        _NC_CACHE = _build_nc()
    nc = _NC_CACHE
    in_maps = _shard_inputs(
        np.asarray(x, np.float32),
        np.asarray(w_qkv, np.float32),
        np.asarray(w_out, np.float32),
    )
    res = run_bass_kernel_spmd(
        nc, in_maps, core_ids=list(range(N_CORES)), trace=trace, tmpdir=tmpdir
    )
    b_out = np.asarray(b_out, np.float32)
    y = np.empty((B, C, L), np.float32)
    for b in range(B):
        y[b] = (
            res.results[2 * b]["out"].astype(np.float32)
            + res.results[2 * b + 1]["out"].astype(np.float32)
            + b_out[:, None]
        )
    return y, res


def kernel(x, w_qkv, w_out, b_out):
    y, _ = _run(x, w_qkv, w_out, b_out, trace=False)
    return y
